# revision 2
# baseline (speedup 1.0000x reference)
"""3-layer GCN (DrugGCN) on 8 Trainium2 NeuronCores via Bass/Tile.

Strategy (node-sharded, dst-partitioned edges):
  - 50000 nodes split into 8 contiguous shards of 6250. Within each core the
    local node columns are padded so every graph's run starts at a multiple of
    8 (pooling windows), giving N_PAD columns per core.
  - Per layer: each core computes z = h @ W for its own nodes (TensorE,
    feature-major h in SBUF), writes z (fp16, node-major) to DRAM, AllGathers
    z across the 8 cores into a Shared DRAM tensor.
  - Edges are owned by the dst core, sorted by (dst block, src half). Edge
    messages are fetched with gpsimd dma_gather (one 256B row per edge) from
    the allgathered z. Scatter-add is a TensorE matmul per 128-edge tile
    against a segment matrix S[e, d] = norm_e * 1[dst_e == d] generated on
    VectorE (iota compare with per-partition scalars), accumulated in PSUM
    feature-major. Self loops are a matmul against a host-built diagonal
    deg_inv matrix. Epilogue relu(+bias) on ScalarE writes the next h.
  - Pooling: window sums/maxes over fixed 8-column windows (one VectorE
    reduce each); the host combines windows into per-graph mean/max.
"""
import os
import sys
import numpy as np

import concourse.bacc as bacc
import concourse.mybir as mybir
import concourse.tile as tile
from concourse.bass_utils import run_bass_kernel_spmd
from concourse.library_config import mlp

NCORES = 8
N = 50000
E = 800000
G = 1600
F = 128
N_LOC = N // NCORES           # 6250
PAD_W = 8                     # pooling window width (columns)
MAX_TILES_PER_GATHER = 32

_CACHE = {}


# ---------------------------------------------------------------- host prep

def _preprocess(edge_index, graph_index):
    src = np.asarray(edge_index[0], dtype=np.int64)
    dst = np.asarray(edge_index[1], dtype=np.int64)
    gi = np.asarray(graph_index, dtype=np.int64)

    deg = np.bincount(dst, minlength=N).astype(np.float64) + 1.0
    deg_isqrt = 1.0 / np.sqrt(deg)
    deg_inv = 1.0 / deg
    norm_e = (deg_isqrt[src] * deg_isqrt[dst]).astype(np.float32)

    # padded column layout per core: graph runs aligned to PAD_W
    col_of = np.zeros(N, dtype=np.int64)       # local padded column
    core_graphs = []                            # per core: list (g, c0, c1)
    npad_c = np.zeros(NCORES, dtype=np.int64)
    for c in range(NCORES):
        lo, hi = c * N_LOC, (c + 1) * N_LOC
        g_loc = gi[lo:hi]
        # run starts (graph boundaries)
        starts = np.flatnonzero(np.r_[True, g_loc[1:] != g_loc[:-1]])
        ends = np.r_[starts[1:], len(g_loc)]
        col = 0
        glist = []
        for s0, s1 in zip(starts, ends):
            col = -(-col // PAD_W) * PAD_W      # align up
            cnt = s1 - s0
            col_of[lo + s0:lo + s1] = col + np.arange(cnt)
            glist.append((int(g_loc[s0]), int(col), int(col + cnt)))
            col += cnt
        core_graphs.append(glist)
        npad_c[c] = col
    n_pad = int(-(-npad_c.max() // 128) * 128)
    assert 4 * n_pad < 32768, f"N_PAD={n_pad} too large for int16 gather idx"
    n_blk = n_pad // 128
    n_win = n_pad // PAD_W

    pad_gid = col_of + (np.arange(N) // N_LOC) * n_pad  # padded global id
    sec_of = pad_gid // (4 * n_pad)                      # 0 or 1
    sec_idx = pad_gid - sec_of * (4 * n_pad)

    ecore = dst // N_LOC
    dcol = col_of[dst]
    dblk = dcol // 128
    din = dcol % 128

    # bucket edges per (core, block, section); fix a deterministic order
    esec = sec_of[src]
    order = np.lexsort((src, esec, dblk, ecore))
    e_sorted = order
    ec_s = ecore[order]
    blk_s = dblk[order]
    sec_s = esec[order]

    counts = np.zeros((NCORES, n_blk, 2), dtype=np.int64)
    np.add.at(counts, (ec_s, blk_s, sec_s), 1)
    cell_tiles = -(-counts.max(axis=0) // 128)          # [n_blk, 2]

    # chunk schedule: consecutive blocks, per-section tiles <= cap
    chunks = []
    b0 = 0
    while b0 < n_blk:
        ta = tb = 0
        b1 = b0
        while b1 < n_blk:
            na, nb = cell_tiles[b1, 0], cell_tiles[b1, 1]
            if b1 > b0 and (ta + na > MAX_TILES_PER_GATHER
                            or tb + nb > MAX_TILES_PER_GATHER):
                break
            ta += na
            tb += nb
            b1 += 1
        chunks.append((b0, b1, int(ta), int(tb)))
        b0 = b1
    msg_slots = max(ta + tb for (_, _, ta, tb) in chunks)

    # tile table order: chunk-major; within chunk sec A tiles (block order)
    # then sec B tiles; slot within chunk == position in this order.
    tile_info = []   # (block, sec, slot_in_chunk, chunk_id)
    for ci, (ba, bb, ta, tb) in enumerate(chunks):
        slot = 0
        for s in (0, 1):
            for b in range(ba, bb):
                for _ in range(int(cell_tiles[b, s])):
                    tile_info.append((b, s, slot, ci))
                    slot += 1
    t_total = len(tile_info)

    # per-core edge slot tables
    gidx = np.zeros((NCORES, 128, t_total * 8), dtype=np.int16)
    dstt = np.zeros((NCORES, 128, t_total), dtype=np.float32)
    normt = np.zeros((NCORES, 128, t_total), dtype=np.float32)

    # per (core, block, sec) -> slice into e_sorted
    cell_start = {}
    pos = 0
    # e_sorted is sorted by (core, block, sec); compute boundaries
    keys = (ec_s * n_blk + blk_s) * 2 + sec_s
    boundaries = np.flatnonzero(np.r_[True, keys[1:] != keys[:-1]])
    b_ends = np.r_[boundaries[1:], len(keys)]
    for bi, be in zip(boundaries, b_ends):
        cell_start[int(keys[bi])] = (int(bi), int(be))

    # tile start offsets per (block, sec) in table order
    tile_cols = {}
    for t, (b, s, slot, ci) in enumerate(tile_info):
        tile_cols.setdefault((b, s), []).append(t)

    idx_flat = np.zeros((NCORES, t_total * 128), dtype=np.int16)
    for c in range(NCORES):
        for (b, s), tcols in tile_cols.items():
            key = (c * n_blk + b) * 2 + s
            if key in cell_start:
                i0, i1 = cell_start[key]
                edges = e_sorted[i0:i1]
            else:
                edges = np.empty(0, dtype=np.int64)
            cnt = len(edges)
            si = sec_idx[src[edges]].astype(np.int16)
            dloc = din[edges].astype(np.float32)
            nv = norm_e[edges]
            for k, t in enumerate(tcols):
                e0, e1 = k * 128, min((k + 1) * 128, cnt)
                n_here = max(0, e1 - e0)
                if n_here > 0:
                    idx_flat[c, t * 128:t * 128 + n_here] = si[e0:e1]
                    dstt[c, :n_here, t] = dloc[e0:e1]
                    normt[c, :n_here, t] = nv[e0:e1]

    # wrap idx into [128, T*8]: idx i -> [16g + i%16, i//16] for g in 0..7
    ar = np.arange(t_total * 128)
    for g in range(8):
        gidx[:, 16 * g + (ar % 16), ar // 16] = idx_flat

    # diagonal deg_inv matrix [128, n_pad] fp16
    dd = np.zeros((NCORES, 128, n_pad), dtype=np.float16)
    node_ids = np.arange(N)
    for c in range(NCORES):
        sel = node_ids[c * N_LOC:(c + 1) * N_LOC]
        cols = col_of[sel]
        p = cols % 128
        dd[c, p, cols] = deg_inv[sel].astype(np.float16)

    sched = dict(
        n_pad=n_pad, n_blk=n_blk, n_win=n_win, t_total=t_total,
        chunks=chunks, tile_info=tile_info, cell_tiles=cell_tiles,
        msg_slots=msg_slots, core_graphs=core_graphs, col_of=col_of,
    )
    tables = dict(gidx=gidx, dstt=dstt, normt=normt, dd=dd)
    return sched, tables


# ---------------------------------------------------------------- program

def _build_program(sched):
    n_pad = sched["n_pad"]
    n_blk = sched["n_blk"]
    n_win = sched["n_win"]
    t_total = sched["t_total"]
    chunks = sched["chunks"]
    tile_info = sched["tile_info"]
    cell_tiles = sched["cell_tiles"]
    msg_slots = sched["msg_slots"]

    f16, f32, i16 = mybir.dt.float16, mybir.dt.float32, mybir.dt.int16

    nc = bacc.Bacc("TRN2", target_bir_lowering=False, debug=False,
                   num_devices=NCORES)

    xT_in = nc.dram_tensor("xT", [128, n_pad], f16, kind="ExternalInput")
    gidx_in = nc.dram_tensor("gidx", [128, t_total * 8], i16, kind="ExternalInput")
    dstt_in = nc.dram_tensor("dstt", [128, t_total], f32, kind="ExternalInput")
    normt_in = nc.dram_tensor("normt", [128, t_total], f32, kind="ExternalInput")
    dd_in = nc.dram_tensor("dd", [128, n_pad], f16, kind="ExternalInput")
    iota_in = nc.dram_tensor("iota", [128, 128], f32, kind="ExternalInput")
    W_in = [nc.dram_tensor(f"W{i}", [128, 128], f16, kind="ExternalInput")
            for i in range(3)]
    b_in = [nc.dram_tensor(f"b{i}", [128, 1], f32, kind="ExternalInput")
            for i in range(3)]
    wsum_out = nc.dram_tensor("wsums", [128, n_win], f32, kind="ExternalOutput")
    wmax_out = nc.dram_tensor("wmaxs", [128, n_win], f32, kind="ExternalOutput")

    z_loc = [nc.dram_tensor(f"z_loc{i}", [n_pad, 128], f16) for i in range(3)]
    z_full = [nc.dram_tensor(f"z_full{i}", [NCORES * n_pad, 128], f16,
                             addr_space="Shared") for i in range(3)]

    # per (block, sec): list of (slot, table_col, chunk)
    blk_tiles = [[[], []] for _ in range(n_blk)]
    for t, (b, s, slot, ci) in enumerate(tile_info):
        blk_tiles[b][s].append((slot, t, ci))
    # gidx column offset of first tile in each chunk/sec
    chunk_gcol = []
    for ci, (ba, bb, ta, tb) in enumerate(chunks):
        a0 = None
        b0 = None
        for t, (b, s, slot, cj) in enumerate(tile_info):
            if cj == ci and s == 0 and a0 is None:
                a0 = t
            if cj == ci and s == 1 and b0 is None:
                b0 = t
        chunk_gcol.append((a0, b0))

    with tile.TileContext(nc) as tc:
        with (
            tc.tile_pool(name="const", bufs=1) as constp,
            tc.tile_pool(name="hbuf", bufs=2) as hpool,
            tc.tile_pool(name="zbuf", bufs=2) as zpool,
            tc.tile_pool(name="msg", bufs=3) as msgpool,
            tc.tile_pool(name="spool", bufs=6) as spool,
            tc.tile_pool(name="zps", bufs=2, space="PSUM") as zpsum,
            tc.tile_pool(name="aggps", bufs=4, space="PSUM") as aggpsum,
            tc.tile_pool(name="outp", bufs=1) as outp,
        ):
            nc.gpsimd.load_library(mlp)

            gidx_sb = constp.tile([128, t_total * 8], i16, tag="gidx")
            nc.sync.dma_start(gidx_sb[:], gidx_in[:])
            dstt_sb = constp.tile([128, t_total], f32, tag="dstt")
            nc.sync.dma_start(dstt_sb[:], dstt_in[:])
            normt_sb = constp.tile([128, t_total], f32, tag="normt")
            nc.sync.dma_start(normt_sb[:], normt_in[:])
            dd_sb = constp.tile([128, n_pad], f16, tag="dd")
            nc.sync.dma_start(dd_sb[:], dd_in[:])
            iota_sb = constp.tile([128, 128], f32, tag="iota")
            nc.sync.dma_start(iota_sb[:], iota_in[:])
            W_sb = []
            b_sb = []
            for i in range(3):
                w = constp.tile([128, 128], f16, tag=f"W{i}")
                nc.sync.dma_start(w[:], W_in[i][:])
                W_sb.append(w)
                b = constp.tile([128, 1], f32, tag=f"b{i}")
                nc.sync.dma_start(b[:], b_in[i][:])
                b_sb.append(b)

            h_cur = hpool.tile([128, n_pad], f16, tag="h")
            nc.sync.dma_start(h_cur[:], xT_in[:])

            relu = mybir.ActivationFunctionType.Relu

            for lay in range(3):
                # ---- z = h @ W (node-major tiles) + stage to DRAM
                z_sb = zpool.tile([128, n_blk, 128], f16, tag="zsb")
                for j in range(n_blk):
                    z_ps = zpsum.tile([128, 128], f32, tag="zps")
                    nc.tensor.matmul(z_ps[:], h_cur[:, j * 128:(j + 1) * 128],
                                     W_sb[lay][:], start=True, stop=True)
                    nc.scalar.copy(z_sb[:, j, :], z_ps[:])
                    nc.sync.dma_start(z_loc[lay][j * 128:(j + 1) * 128, :],
                                      z_sb[:, j, :])
                nc.gpsimd.collective_compute(
                    "AllGather", mybir.AluOpType.bypass,
                    replica_groups=[list(range(NCORES))],
                    ins=[z_loc[lay][:]], outs=[z_full[lay][:]],
                )
                zA = z_full[lay][0:4 * n_pad, :]
                zB = z_full[lay][4 * n_pad:8 * n_pad, :]

                h_next = hpool.tile([128, n_pad], f16, tag="h")

                # ---- edge aggregation per chunk
                for ci, (ba, bb, ta, tb) in enumerate(chunks):
                    msg = msgpool.tile([128, msg_slots, 128], f16, tag="msg")
                    a0, b0 = chunk_gcol[ci]
                    if ta:
                        nc.gpsimd.dma_gather(
                            msg[:, 0:ta, :], zA,
                            gidx_sb[:, a0 * 8:(a0 + ta) * 8],
                            ta * 128, ta * 128, 128, single_packet=False)
                    if tb:
                        nc.gpsimd.dma_gather(
                            msg[:, ta:ta + tb, :], zB,
                            gidx_sb[:, b0 * 8:(b0 + tb) * 8],
                            tb * 128, tb * 128, 128, single_packet=False)
                    for j in range(ba, bb):
                        tiles = blk_tiles[j][0] + blk_tiles[j][1]
                        agg = aggpsum.tile([128, 128], f32, tag="agg")
                        nc.tensor.matmul(agg[:], z_sb[:, j, :],
                                         dd_sb[:, j * 128:(j + 1) * 128],
                                         start=True, stop=(len(tiles) == 0))
                        for k, (slot, tcol, _) in enumerate(tiles):
                            s_t = spool.tile([128, 128], f16, tag="S")
                            nc.vector.tensor_scalar(
                                s_t[:], iota_sb[:],
                                dstt_sb[:, tcol:tcol + 1],
                                normt_sb[:, tcol:tcol + 1],
                                mybir.AluOpType.is_equal, mybir.AluOpType.mult)
                            nc.tensor.matmul(agg[:], msg[:, slot, :], s_t[:],
                                             start=False,
                                             stop=(k == len(tiles) - 1))
                        nc.scalar.activation(
                            h_next[:, j * 128:(j + 1) * 128], agg[:],
                            relu, bias=b_sb[lay][:])
                h_cur = h_next

            # ---- pooling: window sums / maxes
            ws_sb = outp.tile([128, n_win], f32, tag="ws")
            wm_sb = outp.tile([128, n_win], f32, tag="wm")
            h3 = h_cur[:].rearrange("p (w k) -> p w k", k=PAD_W)
            nc.vector.tensor_reduce(ws_sb[:], h3, mybir.AxisListType.X,
                                    mybir.AluOpType.add)
            nc.vector.tensor_reduce(wm_sb[:], h3, mybir.AxisListType.X,
                                    mybir.AluOpType.max)
            nc.sync.dma_start(wsum_out[:], ws_sb[:])
            nc.sync.dma_start(wmax_out[:], wm_sb[:])

    nc.compile()
    return nc


# ---------------------------------------------------------------- kernel

def kernel(x, edge_index, graph_index, W1, b1, W2, b2, W3, b3):
    key = "gcn"
    if key not in _CACHE:
        sched, tables = _preprocess(edge_index, graph_index)
        nc = _build_program(sched)
        _CACHE[key] = (sched, tables, nc)
    sched, tables, nc = _CACHE[key]

    n_pad = sched["n_pad"]
    col_of = sched["col_of"]
    n_win = sched["n_win"]

    x = np.asarray(x, dtype=np.float32)
    Ws = [np.asarray(w, dtype=np.float32) for w in (W1, W2, W3)]
    bs = [np.asarray(b, dtype=np.float32) for b in (b1, b2, b3)]

    iota = np.tile(np.arange(128, dtype=np.float32), (128, 1))
    in_maps = []
    for c in range(NCORES):
        sel = np.arange(c * N_LOC, (c + 1) * N_LOC)
        xT = np.zeros((128, n_pad), dtype=np.float16)
        xT[:, col_of[sel]] = x[sel].T.astype(np.float16)
        m = {
            "xT": xT,
            "gidx": tables["gidx"][c],
            "dstt": tables["dstt"][c],
            "normt": tables["normt"][c],
            "dd": tables["dd"][c],
            "iota": iota,
        }
        for i in range(3):
            m[f"W{i}"] = Ws[i].astype(np.float16)
            m[f"b{i}"] = bs[i].reshape(128, 1)
        in_maps.append(m)

    res = run_bass_kernel_spmd(nc, in_maps, list(range(NCORES)))
    return _combine(res.results, sched, graph_index)


def _combine(results, sched, graph_index):
    gi = np.asarray(graph_index, dtype=np.int64)
    counts = np.bincount(gi, minlength=G).astype(np.float64)
    sums = np.zeros((G, F), dtype=np.float64)
    maxs = np.full((G, F), -np.inf, dtype=np.float64)
    for c in range(NCORES):
        ws = results[c]["wsums"].astype(np.float64)   # [128, n_win]
        wm = results[c]["wmaxs"]
        for (g, c0, c1) in sched["core_graphs"][c]:
            w0, w1 = c0 // PAD_W, -(-c1 // PAD_W)
            sums[g] += ws[:, w0:w1].sum(axis=1)
            maxs[g] = np.maximum(maxs[g], wm[:, w0:w1].max(axis=1))
    mean = sums / np.maximum(counts, 1.0)[:, None]
    out = np.concatenate([mean, maxs], axis=-1).astype(np.float32)
    return out
